# revision 26
# baseline (speedup 1.0000x reference)
"""Trainium2 Bass kernel for nn_BiGNN_53772990546511 (v2, restructured).

Same math identities as the validated baseline (relu(elu)=relu; days 2..4
collapse to row algebra; day-1 attention is a 1024x1024 users-x-locs problem
with a {0,1,2} multiplicity mask; exp(leaky(E)) = max(exp(E-10),
exp(0.2E-10)); x_user = A_hat @ x_loc with A_hat host-built from indices).

v2 changes, driven by the baseline profile (136us, PE cold 50% of the run,
15us dead start, ACT 26us serial exp, 34us of fp32 LOW_HIGH matmuls):
  - all inputs pre-tiled host-side into 4 contiguous f16 DMAs (smalls,
    bundle, AhatT, MT) ordered by need-time; everything on-device is f16
    except f32 PSUM accumulation and f32 output staging.
  - exp(0.2E) branch is rank-1: exp(0.2f1-5) (x) exp(0.2f2-5). Only vector
    exps on ACT; the matrix branch2 is one 4x tensor_scalar per tile.
    Branch1 stays on ACT (exp with per-partition f2 bias). max+mask on DVE.
  - f1 is obtained for free as a 257th column of the x_user matmul
    (moving operand carries xw1 = xloc @ (W a1)).
  - attention accumulates 8 user-tile PSUM banks across loc tiles as PT
    tiles are produced (ACT/DVE/PE pipeline); Z is the 257th whext column.
  - one PSUM pool (8 bufs x 1 bank) rotated through all phases.
  - junk warm-up matmuls keep the PE HAM un-throttled through the input
    DMA window; day-0 loc block is written from SBUF (f16->f32 cast)
    instead of a DRAM->DRAM copy.

Sharding: unchanged -- 8 cores = 4 batch pairs, both cores of a pair run the
full per-batch recurrence; odd cores get user-rotated (by 512) index
tensors and each writes half of the user rows plus one of the two
(identical) loc blocks.
"""
import numpy as np

N_USER = 1024
N_LOC = 1024
DM = 256
HD = 256
B = 4
D = 5
ALPHA = 0.2
P = 128
NCORES = 8

# bundle free-dim offsets (f16 words)
OFF_XLT = 0              # xlocT16  [128, 2*1024]
OFF_XLE = 2048           # xloc16e  [128, 8*257]
OFF_W = OFF_XLE + 8 * 257       # W16   [128, 2*256]
OFF_WT = OFF_W + 512            # WT16  [128, 2*256]
OFF_AC = OFF_WT + 512           # acol16 [128, 4]
OFF_UW = OFF_AC + 4             # uw3   [128, 8*3]
NB = OFF_UW + 24

# smalls [3, *] free-dim offsets (day-indexed data lives on partition 0,
# because matmul operands need base partition 0/32/64)
SM_NOTHAS = 0            # [0:1, 0:1024]
SM_HASR = 1024           # [0:1, 1024 + dd*1024 : ...]  (3 days)
SM_NOTHASR = 4096        # [0:1, 4096 + dd*1024 : ...]
SM_NER = 7168            # [0:1, 7168 + dd*2 : +2]
SM_HCOL = 7174           # [0:3, 7174:7175]
SM_CPAIR = 7175          # [0:2, 7175:7176]
NS = 7176

_CACHE = {}


def _apply_tile_patch():
    import concourse.tile as tile
    from concourse.tile_sem_assignment import tick_to_sem

    if not getattr(tile.TileContext, "_drain_patched", False):
        def _patched(self, tick_clock, wait_clock):
            nc = self.nc
            gc = tick_clock.global_clock
            for proc, sem in self.sems.allocated().items():
                t = gc[proc]
                if t and t > 0:
                    nc.sync.nop().wait_op(sem, tick_to_sem(t, proc), "sem-ge")
            nc.sync.drain()
            nc.all_engine_barrier()
            popped = nc._tile_sem_poison_stack.pop()
            assert popped is self._sem_poison
            nc.clear_and_free_semaphores(list(self.sems.allocated().values()))
            nc.all_engine_barrier()

        tile.TileContext._drain_and_barrier = _patched
        tile.TileContext._drain_patched = True

    import json as _json
    import concourse.bass_utils as _bu
    import concourse.bass2jax as _b2j

    if not getattr(_bu, "_wait_split_patched", False):
        _orig_compile = _bu.compile_bir_kernel

        def _split_waits(bir_json):
            j = _json.loads(bir_json)
            nid = [0]
            for fn in j.get("functions", []):
                for bb in fn.get("blocks", []):
                    out = []
                    for inst in bb.get("instructions", []):
                        si = inst.get("sync_info") or {}
                        ow = si.get("on_wait") or []
                        if len(ow) > 1:
                            for w in ow[:-1]:
                                nid[0] += 1
                                out.append({
                                    "debug": inst.get("debug", 0),
                                    "engine": inst.get("engine", "SP"),
                                    "ins": [],
                                    "name": f"WSPL-{nid[0]}",
                                    "opcode": "NoOp",
                                    "outs": [],
                                    "sync_info": {"on_update": [],
                                                  "on_wait": [w]},
                                })
                            si["on_wait"] = [ow[-1]]
                        out.append(inst)
                    bb["instructions"] = out
            return _json.dumps(j).encode()

        def _patched_compile(bir_json, tmpdir, neff_name="file.neff"):
            return _orig_compile(_split_waits(bir_json), tmpdir,
                                 neff_name=neff_name)

        _bu.compile_bir_kernel = _patched_compile
        _b2j.compile_bir_kernel = _patched_compile
        _bu._wait_split_patched = True


def _build_nc():
    import contextlib
    import concourse.bass as bass
    import concourse.tile as tile
    from concourse import mybir

    _apply_tile_patch()
    f32 = mybir.dt.float32
    f16 = mybir.dt.float16
    AF = mybir.ActivationFunctionType
    OP = mybir.AluOpType

    nc = bass.Bass()

    d_small = nc.dram_tensor("smalls", [3, NS], f16, kind="ExternalInput")
    d_bund = nc.dram_tensor("bund", [P, NB], f16, kind="ExternalInput")
    d_Ah = nc.dram_tensor("AhT", [P, 8 * N_USER], f16, kind="ExternalInput")
    d_MT = nc.dram_tensor("MTt", [P, 8 * N_USER], f16, kind="ExternalInput")
    d_out = nc.dram_tensor("out", [D, 1536, HD], f32, kind="ExternalOutput")

    with tile.TileContext(nc) as tc:
        with contextlib.ExitStack() as ctx:
            persist = ctx.enter_context(tc.tile_pool(name="persist", bufs=1))
            work = ctx.enter_context(tc.tile_pool(name="work", bufs=1))
            ps = ctx.enter_context(
                tc.tile_pool(name="ps", bufs=8, space="PSUM"))

            def pst(nm, part=P, width=512):
                return ps.tile([part, width], f32, name=nm, tag="pb")

            # ---------------- consts + warmup ----------------
            onesrow16 = persist.tile([1, P], f16, name="onesrow16")
            nc.vector.memset(onesrow16[:], 1.0)
            onescol16 = persist.tile([P, 1], f16, name="onescol16")
            nc.vector.memset(onescol16[:], 1.0)
            one11_16 = persist.tile([1, 1], f16, name="one11_16")
            nc.vector.memset(one11_16[:], 1.0)
            l3 = persist.tile([P, 3], f16, name="l3")
            nc.vector.memset(l3[:, 0:2], 0.0)
            nc.vector.memset(l3[:, 2:3], 1.0)
            bm10 = persist.tile([P, 1], f32, name="bm10")
            nc.vector.memset(bm10[:], -10.0)
            bm5 = persist.tile([P, 1], f32, name="bm5")
            nc.vector.memset(bm5[:], -5.0)
            # ---------------- input DMAs (sync HWDGE, in order) ----------
            bund = persist.tile([P, NB], f16, name="bund")
            nc.sync.dma_start(out=bund[:], in_=d_bund[:])
            AhT = persist.tile([P, 8 * N_USER], f16, name="AhT")
            MTt = persist.tile([P, 8 * N_USER], f16, name="MTt")
            H = 4 * N_USER
            nc.sync.dma_start(out=AhT[:, 0:H], in_=d_Ah[:, 0:H])
            nc.sync.dma_start(out=AhT[:, H:2 * H], in_=d_Ah[:, H:2 * H])
            nc.sync.dma_start(out=MTt[:, 0:H], in_=d_MT[:, 0:H])
            nc.sync.dma_start(out=MTt[:, H:2 * H], in_=d_MT[:, H:2 * H])
            smalls = persist.tile([3, NS], f16, name="smalls")
            nc.sync.dma_start(out=smalls[:], in_=d_small[:])

            def xlt(kd, sl):
                return bund[:, OFF_XLT + kd * N_LOC:OFF_XLT + (kd + 1) * N_LOC][:, sl]

            def xle(lt, w=257):
                return bund[:, OFF_XLE + lt * 257:OFF_XLE + lt * 257 + w]

            def W16(kd):
                return bund[:, OFF_W + kd * HD:OFF_W + (kd + 1) * HD]

            def WT16(kd):
                return bund[:, OFF_WT + kd * DM:OFF_WT + (kd + 1) * DM]

            W16c16 = persist.tile([P, 4 * HD], f16, name="W16c16")

            def W16c(j):
                return W16c16[:, j * HD:(j + 1) * HD]

            acol16 = bund[:, OFF_AC:OFF_AC + 4]
            nothas = smalls[0:1, SM_NOTHAS:SM_NOTHAS + N_USER]

            # ---------------- phase 1: small matmuls off the bundle ------
            for kd in range(2):
                nc.vector.tensor_scalar(out=W16c(kd), in0=W16(kd),
                                        scalar1=2048.0 / 3072.0,
                                        scalar2=None, op0=OP.mult)
                nc.vector.tensor_scalar(out=W16c(2 + kd), in0=W16(kd),
                                        scalar1=1.0 / 3072.0,
                                        scalar2=None, op0=OP.mult)
            wacols_ps = pst("wacols")
            for j in range(4):          # j = ai*2 + dm-chunk
                ai, mt = j // 2, j % 2
                for kh in range(2):
                    nc.tensor.matmul(
                        wacols_ps[:, j:j + 1],
                        WT16(kh)[:, mt * P:(mt + 1) * P],
                        acol16[:, ai * 2 + kh:ai * 2 + kh + 1],
                        start=(kh == 0), stop=(kh == 1))
            wa16 = persist.tile([P, 4], f16, name="wa16")
            nc.scalar.copy(wa16[:], wacols_ps[:, 0:4])

            xwps = pst("xwps")
            for lt in range(8):
                sl = slice(lt * P, (lt + 1) * P)
                for kd in range(2):
                    nc.tensor.matmul(xwps[:, lt:lt + 1], xlt(kd, sl),
                                     wa16[:, kd:kd + 1],
                                     start=(kd == 0), stop=(kd == 1))
                for kd in range(2):
                    nc.tensor.matmul(xwps[:, 8 + lt:9 + lt], xlt(kd, sl),
                                     wa16[:, 2 + kd:3 + kd],
                                     start=(kd == 0), stop=(kd == 1))
            # insert xw1 columns into the 257-wide xloc tiles
            nc.scalar.copy(
                bund[:, OFF_XLE:OFF_XLE + 8 * 257]
                .rearrange("p (l w) -> p l w", l=8)[:, :, 256:257]
                .rearrange("p l w -> p (l w)"),
                xwps[:, 0:8])
            fb1c = persist.tile([P, 8], f32, name="fb1c")
            nc.scalar.activation(fb1c[:], xwps[:, 8:16], AF.Identity,
                                 bias=bm10[:])
            dcols = persist.tile([P, 8], f32, name="dcols")
            nc.scalar.activation(dcols[:], xwps[:, 8:16], AF.Exp,
                                 bias=bm5[:], scale=ALPHA)
            fb2c = persist.tile([P, 8], f32, name="fb2c")
            nc.scalar.activation(fb2c[:], xwps[:, 8:16], AF.Identity,
                                 bias=bm10[:], scale=ALPHA)

            whext = persist.tile([P, 8 * 257], f16, name="whext")
            nc.vector.memset(
                whext[:].rearrange("p (l w) -> p l w", l=8)[:, :, 256:257],
                1.0)
            for lt in range(8):
                sl = slice(lt * P, (lt + 1) * P)
                whp = pst(f"wh{lt}")
                for kd in range(2):
                    nc.tensor.matmul(whp[:, 0:HD], xlt(kd, sl), W16(kd),
                                     start=(kd == 0), stop=(kd == 1))
                nc.vector.tensor_copy(whext[:, lt * 257:lt * 257 + HD],
                                      whp[:, 0:HD])

            # day-0 loc block staging (f16 -> f32); DMA is issued after the
            # MT load so the 1MB write does not delay the mask transfer.
            xlocf32 = persist.tile([P, 8 * DM], f32, name="xlocf32")
            nc.vector.tensor_copy(
                xlocf32[:].rearrange("p (l w) -> p l w", l=8),
                bund[:, OFF_XLE:OFF_XLE + 8 * 257]
                .rearrange("p (l w) -> p l w", l=8)[:, :, 0:256])
            nc.sync.dma_start(
                out=d_out[0, 512:1536, :].rearrange("(t p) h -> p t h", p=P),
                in_=xlocf32[:].rearrange("p (t h) -> p t h", t=8))

            # ---------------- phase 2: x_user (+f1 column) ---------------
            # F1B built directly: stationary xw1 column broadcast across all
            # 128 array columns (stride-0 free dim) -> every out partition
            # gets f1_u = sum_l xw1_l * AhT[l, u]. No transposes, no f1row.
            xw1c16 = persist.tile([P, 8], f16, name="xw1c16")
            nc.scalar.copy(xw1c16[:], xwps[:, 0:8])
            F1B16 = persist.tile([P, N_USER], f16, name="F1B16")
            for c in range(2):
                fbp = pst(f"fbp{c}")
                for lt in range(8):
                    w = xw1c16[:, lt:lt + 1]
                    wb = bass.AP(tensor=w.tensor, offset=w.offset,
                                 ap=[list(w.ap[0]), [0, P]])
                    nc.tensor.matmul(
                        fbp[:],
                        wb,
                        AhT[:, lt * N_USER + c * 512:lt * N_USER + (c + 1) * 512],
                        start=(lt == 0), stop=(lt == 7))
                nc.vector.tensor_copy(F1B16[:, c * 512:(c + 1) * 512], fbp[:])
            E2B16 = persist.tile([P, N_USER], f16, name="E2B16")
            nc.scalar.activation(E2B16[:], F1B16[:], AF.Exp,
                                 bias=bm5[:], scale=ALPHA)

            xu_all = persist.tile([P, 4 * DM], f32, name="xu_all")
            xu16 = persist.tile([P, 8 * DM], f16, name="xu16")
            for ut in range(8):
                xp = pst(f"xu{ut}")
                for lt in range(8):
                    nc.tensor.matmul(
                        xp[:, 0:DM],
                        AhT[:, lt * N_USER + ut * P:lt * N_USER + (ut + 1) * P],
                        xle(lt, 256), start=(lt == 0), stop=(lt == 7))
                nc.vector.tensor_copy(xu16[:, ut * DM:(ut + 1) * DM],
                                      xp[:, 0:DM])


            # ---------------- phase 3: means / day-0 ---------------------
            stp = pst("st3", part=3)
            for ut in range(8):
                nc.tensor.matmul(stp[0:3, 0:DM],
                                 bund[:, OFF_UW + ut * 3:OFF_UW + (ut + 1) * 3],
                                 xu16[:, ut * DM:(ut + 1) * DM],
                                 start=(ut == 0), stop=False)
            for lt in range(8):
                nc.tensor.matmul(stp[0:3, 0:DM], l3[:], xle(lt, 256),
                                 start=False, stop=(lt == 7))
            stack16 = persist.tile([3, DM], f16, name="stack16")
            nc.scalar.copy(stack16[:], stp[0:3, 0:DM])
            mwe16 = stack16[0:1, :]

            # ---------------- phase 4: mw0 -------------------------------
            hcol16 = smalls[0:3, SM_HCOL:SM_HCOL + 1]
            mcp = pst("mcp")
            for mt in range(2):
                nc.tensor.matmul(mcp[:, mt:mt + 1],
                                 stack16[:, mt * P:(mt + 1) * P], hcol16,
                                 start=True, stop=True)
            mc16 = persist.tile([P, 2], f16, name="mc16")
            nc.scalar.copy(mc16[:], mcp[:, 0:2])
            mwp = pst("mwp", part=1)
            for kd in range(2):
                nc.tensor.matmul(mwp[0:1, 0:HD], mc16[:, kd:kd + 1], W16(kd),
                                 start=(kd == 0), stop=(kd == 1))
            mw0e16 = persist.tile([1, 257], f16, name="mw0e16")
            nc.vector.memset(mw0e16[:, 256:257], 1.0)
            nc.scalar.copy(mw0e16[:, 0:HD], mwp[0:1, 0:HD])
            # v1 = relu(mw0); early day-1 loc broadcast
            v1row16 = persist.tile([1, HD], f16, name="v1row16")
            nc.scalar.activation(v1row16[:], mwp[0:1, 0:HD], AF.Relu)

            def bcast_loc(day, vrow16, nm):
                bcp = pst(f"bc{nm}")
                nc.tensor.matmul(bcp[:, 0:HD], onesrow16[:], vrow16[:],
                                 start=True, stop=True)
                vst = work.tile([P, HD], f32, name=f"vst{nm}", tag="vst",
                                bufs=4)
                nc.vector.tensor_copy(vst[:], bcp[:, 0:HD])
                ap = vst[:]
                src = bass.AP(tensor=ap.tensor, offset=ap.offset,
                              ap=[list(ap.ap[0]), [0, 8], list(ap.ap[1])])
                dst = d_out[day, 512:1536, :].rearrange("(t p) h -> p t h", p=P)
                nc.sync.dma_start(out=dst, in_=src)

            bcast_loc(1, v1row16, "d1")

            # ---------------- phase 5+6: e-stage + attention pipeline ----
            PT = persist.tile([P, 8 * N_USER], f16, name="PT")
            att = [pst(f"att{ut}") for ut in range(8)]
            for ut in range(8):
                nc.tensor.matmul(att[ut][:, 0:257],
                                 nothas[:, ut * P:(ut + 1) * P], mw0e16[:],
                                 start=True, stop=False)
            for pair in range(4):
                x1 = work.tile([P, 2 * N_USER], f16, name="x1", tag="x1",
                               bufs=3)
                t2 = work.tile([P, 2 * N_USER], f16, name="t2", tag="t2",
                               bufs=2)
                for h in range(2):
                    lt = 2 * pair + h
                    nc.scalar.activation(
                        x1[:, h * N_USER:(h + 1) * N_USER], F1B16[:], AF.Exp,
                        bias=fb1c[:, lt:lt + 1])
                    if pair < 2:
                        # ACT is idle early in the e-stage window; let it
                        # compute branch2 directly for the first tiles.
                        nc.scalar.activation(
                            t2[:, h * N_USER:(h + 1) * N_USER], F1B16[:],
                            AF.Exp, bias=fb2c[:, lt:lt + 1], scale=ALPHA)
                    else:
                        nc.vector.tensor_scalar(
                            out=t2[:, h * N_USER:(h + 1) * N_USER],
                            in0=E2B16[:], scalar1=dcols[:, lt:lt + 1],
                            scalar2=None, op0=OP.mult)
                mx = work.tile([P, 2 * N_USER], f16, name="mx", tag="mx",
                               bufs=2)
                nc.vector.tensor_tensor(out=mx[:], in0=x1[:], in1=t2[:],
                                        op=OP.max)
                nc.vector.tensor_tensor(
                    out=PT[:, 2 * pair * N_USER:(2 * pair + 2) * N_USER],
                    in0=mx[:],
                    in1=MTt[:, 2 * pair * N_USER:(2 * pair + 2) * N_USER],
                    op=OP.mult)
                for h in range(2):
                    lt = 2 * pair + h
                    for ut in range(8):
                        nc.tensor.matmul(
                            att[ut][:, 0:257],
                            PT[:, lt * N_USER + ut * P:lt * N_USER + (ut + 1) * P],
                            whext[:, lt * 257:(lt + 1) * 257],
                            start=False, stop=(lt == 7))

            # finalize h1 = relu(numer/Z)
            h1u = persist.tile([P, 4 * DM], f32, name="h1u")
            h1u16 = persist.tile([P, 8 * DM], f16, name="h1u16")
            for ut in range(8):
                zr = work.tile([P, 1], f32, name="zr", tag="zr", bufs=3)
                nc.vector.reciprocal(zr[:], att[ut][:, 256:257])
                nc.scalar.activation(h1u16[:, ut * DM:(ut + 1) * DM],
                                     att[ut][:, 0:DM], AF.Relu, scale=zr[:])
                if ut < 4:
                    nc.vector.tensor_scalar(
                        out=h1u[:, ut * DM:(ut + 1) * DM],
                        in0=att[ut][:, 0:DM], scalar1=zr[:], scalar2=0.0,
                        op0=OP.mult, op1=OP.max)
            nc.sync.dma_start(
                out=d_out[1, 0:512, :].rearrange("(t p) h -> p t h", p=P),
                in_=h1u[:].rearrange("p (t h) -> p t h", t=4))

            # day-0 completion (off the critical path): stage f32 users,
            # add the nothas (x) mwe fallback, write day-0 user rows.
            nc.vector.tensor_copy(xu_all[:], xu16[:, 0:4 * DM])
            for c in range(2):
                mbp = pst(f"mb{c}")
                for h in range(2):
                    nc.tensor.matmul(
                        mbp[:, h * DM:(h + 1) * DM],
                        nothas[:, (2 * c + h) * P:(2 * c + h + 1) * P],
                        mwe16, start=True, stop=True)
                nc.vector.tensor_tensor(
                    out=xu_all[:, c * 512:(c + 1) * 512],
                    in0=xu_all[:, c * 512:(c + 1) * 512],
                    in1=mbp[:], op=OP.add)
            nc.sync.dma_start(
                out=d_out[0, 0:512, :].rearrange("(t p) h -> p t h", p=P),
                in_=xu_all[:].rearrange("p (t h) -> p t h", t=4))

            # ---------------- phase 7: days 2..4 row algebra -------------
            # vs16 cols: [v_kd0, s_kd0, v_kd1, s_kd1]
            vsp = pst("vs1")
            for mt in range(2):
                nc.tensor.matmul(vsp[:, 2 * mt:2 * mt + 1],
                                 v1row16[0:1, mt * P:(mt + 1) * P],
                                 one11_16[:], start=True, stop=True)
                for ut in range(8):
                    nc.tensor.matmul(
                        vsp[:, 2 * mt + 1:2 * mt + 2],
                        h1u16[:, ut * DM + mt * P:ut * DM + (mt + 1) * P],
                        onescol16[:], start=(ut == 0), stop=(ut == 7))
            vs16 = work.tile([P, 4], f16, name="vs16_1", tag="vs16", bufs=2)
            nc.scalar.copy(vs16[:], vsp[:, 0:4])

            for day in (2, 3, 4):
                dd = day - 2
                outu = work.tile([P, 4 * DM], f32, name=f"outu{day}",
                                 tag="outu", bufs=3)
                skp = pst(f"stk{day}", part=1)
                for kd in range(2):
                    nc.tensor.matmul(skp[0:1, 0:HD],
                                     vs16[:, 2 * kd:2 * kd + 1], W16(kd),
                                     start=(kd == 0), stop=(kd == 1))
                mrp = pst(f"mr{day}", part=1)
                for kd in range(2):
                    nc.tensor.matmul(mrp[0:1, 0:HD],
                                     vs16[:, 2 * kd:2 * kd + 1], W16c(kd),
                                     start=(kd == 0), stop=False)
                    nc.tensor.matmul(mrp[0:1, 0:HD],
                                     vs16[:, 2 * kd + 1:2 * kd + 2],
                                     W16c(2 + kd),
                                     start=False, stop=(kd == 1))
                r1_16 = work.tile([1, HD], f16, name=f"r1_{day}", tag="r1",
                                  bufs=2)
                nc.scalar.activation(r1_16[:], skp[0:1, 0:HD], AF.Relu)
                vn16 = work.tile([1, HD], f16, name=f"vn{day}", tag="vn",
                                 bufs=2)
                nc.scalar.activation(vn16[:], mrp[0:1, 0:HD], AF.Relu)
                bcast_loc(day, vn16, f"d{day}")
                for ut in range(4):
                    oup = pst(f"ou{day}_{ut}")
                    usl = slice(SM_HASR + dd * N_USER + ut * P,
                                SM_HASR + dd * N_USER + (ut + 1) * P)
                    nsl = slice(SM_NOTHASR + dd * N_USER + ut * P,
                                SM_NOTHASR + dd * N_USER + (ut + 1) * P)
                    nc.tensor.matmul(oup[:, 0:HD],
                                     smalls[0:1, usl], r1_16[:],
                                     start=True, stop=False)
                    nc.tensor.matmul(oup[:, 0:HD],
                                     smalls[0:1, nsl], vn16[:],
                                     start=False, stop=True)
                    nc.vector.tensor_copy(outu[:, ut * DM:(ut + 1) * DM],
                                          oup[:, 0:HD])
                nc.sync.dma_start(
                    out=d_out[day, 0:512, :].rearrange("(t p) h -> p t h", p=P),
                    in_=outu[:].rearrange("p (t h) -> p t h", t=4))
                if day < 4:
                    vnp = pst(f"vn{day}")
                    for mt in range(2):
                        nc.tensor.matmul(vnp[:, 2 * mt:2 * mt + 1],
                                         vn16[0:1, mt * P:(mt + 1) * P],
                                         one11_16[:], start=True, stop=True)
                        nc.tensor.matmul(
                            vnp[:, 2 * mt + 1:2 * mt + 2],
                            r1_16[0:1, mt * P:(mt + 1) * P],
                            smalls[0:1, SM_NER + 2 * dd:SM_NER + 2 * dd + 1],
                            start=True, stop=False)
                        nc.tensor.matmul(
                            vnp[:, 2 * mt + 1:2 * mt + 2],
                            vn16[0:1, mt * P:(mt + 1) * P],
                            smalls[0:1,
                                   SM_NER + 2 * dd + 1:SM_NER + 2 * dd + 2],
                            start=False, stop=True)
                    vs16 = work.tile([P, 4], f16, name=f"vs16_{day}",
                                     tag="vs16", bufs=2)
                    nc.scalar.copy(vs16[:], vnp[:, 0:4])
    return nc


def _tile128(x):
    r, c = x.shape
    return np.ascontiguousarray(
        x.reshape(r // P, P, c).transpose(1, 0, 2).reshape(P, -1))


def _host_prep(x_loc, mob_links, text_links, W, a):
    x_loc = np.asarray(x_loc, np.float32)
    W = np.asarray(W, np.float32)
    a = np.asarray(a, np.float32).reshape(-1)
    mob = np.asarray(mob_links)
    text = np.asarray(text_links)

    xe = np.zeros((N_LOC, 257), np.float32)
    xe[:, :256] = x_loc
    bund_shared = np.concatenate([
        _tile128(np.ascontiguousarray(x_loc.T)),
        _tile128(xe),
        _tile128(W),
        _tile128(np.ascontiguousarray(W.T)),
        a.reshape(2, 2, P).transpose(2, 0, 1).reshape(P, 4),
        np.zeros((P, 24), np.float32),   # per-core uw3 patched below
    ], axis=1).astype(np.float16)

    in_maps = []
    for c in range(NCORES):
        b, r = c // 2, c % 2
        rot = r * 512
        u0 = np.concatenate([mob[b, 0, :, 0], text[b, 0, :, 0]]).astype(np.int64)
        l0 = np.concatenate([mob[b, 0, :, 1], text[b, 0, :, 1]]).astype(np.int64)
        cnt = np.bincount(u0, minlength=N_USER).astype(np.float32)
        A = np.zeros((N_USER, N_LOC), np.float32)
        np.add.at(A, (u0, l0), 1.0)
        Ahat = A / np.maximum(cnt, 1.0)[:, None]
        Mb = np.zeros((N_USER, N_LOC), np.float32)
        Tb = np.zeros((N_USER, N_LOC), np.float32)
        Mb[mob[b, 0, :, 0], mob[b, 0, :, 1]] = 1.0
        Tb[text[b, 0, :, 0], text[b, 0, :, 1]] = 1.0
        M = Mb + Tb
        has0 = (cnt > 0).astype(np.float32)
        n_with = max(float(has0.sum()), 1.0)
        nh_cnt = float(N_USER) - float(has0.sum())

        def rollu(x, axis=0):
            return np.roll(x, -rot, axis=axis)

        hasE = np.zeros((3, N_USER), np.float32)
        for dd in range(3):
            us = np.concatenate([mob[b, dd + 1, :, 0], text[b, dd + 1, :, 0]])
            hasE[dd, us] = 1.0
        hasr = np.stack([rollu(hasE[dd]) for dd in range(3)])
        ner = np.stack([np.array([hasE[dd].sum(), N_USER - hasE[dd].sum()],
                                 np.float32) for dd in range(3)])
        nothas_r = rollu(1.0 - has0)
        hw = (rollu(has0) / n_with).astype(np.float32)
        uw3 = np.stack([hw, np.ones(N_USER, np.float32),
                        np.zeros(N_USER, np.float32)], axis=1)

        smalls = np.zeros((3, NS), np.float32)
        smalls[0, SM_NOTHAS:SM_NOTHAS + N_USER] = nothas_r
        smalls[0, SM_HASR:SM_HASR + 3 * N_USER] = hasr.reshape(-1)
        smalls[0, SM_NOTHASR:SM_NOTHASR + 3 * N_USER] = \
            (1.0 - hasr).reshape(-1)
        smalls[0, SM_NER:SM_NER + 6] = ner.reshape(-1)
        smalls[0:3, SM_HCOL:SM_HCOL + 1] = np.array(
            [[nh_cnt / 3072.0], [1.0 / 3072.0], [2.0 / 3072.0]], np.float32)
        smalls[0:2, SM_CPAIR:SM_CPAIR + 1] = np.array(
            [[2048.0 / 3072.0], [1.0 / 3072.0]], np.float32)

        bund = bund_shared.copy()
        bund[:, OFF_UW:OFF_UW + 24] = _tile128(rollu(uw3)).astype(np.float16)
        m = {
            "smalls": smalls.astype(np.float16),
            "bund": bund,
            "AhT": _tile128(
                np.ascontiguousarray(rollu(Ahat, 0).T)).astype(np.float16),
            "MTt": _tile128(
                np.ascontiguousarray(rollu(M, 0).T)).astype(np.float16),
        }
        in_maps.append(m)
    return in_maps


def kernel(**inputs):
    from concourse.bass_utils import run_bass_kernel_spmd

    if "nc" not in _CACHE:
        _CACHE["nc"] = _build_nc()
    nc = _CACHE["nc"]

    in_maps = _host_prep(inputs["x_loc"], inputs["mob_links"],
                         inputs["text_links"], inputs["W"], inputs["a"])
    res = run_bass_kernel_spmd(nc, in_maps, core_ids=list(range(NCORES)))

    out = np.zeros((B, D, N_USER + 2 * N_LOC, HD), np.float32)
    for c in range(NCORES):
        b, r = c // 2, c % 2
        o = res.results[c]["out"]
        out[b, :, r * 512:(r + 1) * 512, :] = o[:, 0:512, :]
        out[b, :, N_USER + r * N_LOC:N_USER + (r + 1) * N_LOC, :] = \
            o[:, 512:1536, :]
    return out


# revision 27
# speedup vs baseline: 1.0147x; 1.0147x over previous
"""Trainium2 Bass kernel for nn_BiGNN_53772990546511 (v2, restructured).

Same math identities as the validated baseline (relu(elu)=relu; days 2..4
collapse to row algebra; day-1 attention is a 1024x1024 users-x-locs problem
with a {0,1,2} multiplicity mask; exp(leaky(E)) = max(exp(E-10),
exp(0.2E-10)); x_user = A_hat @ x_loc with A_hat host-built from indices).

v2 changes, driven by the baseline profile (136us, PE cold 50% of the run,
15us dead start, ACT 26us serial exp, 34us of fp32 LOW_HIGH matmuls):
  - all inputs pre-tiled host-side into 4 contiguous f16 DMAs (smalls,
    bundle, AhatT, MT) ordered by need-time; everything on-device is f16
    except f32 PSUM accumulation and f32 output staging.
  - exp(0.2E) branch is rank-1: exp(0.2f1-5) (x) exp(0.2f2-5). Only vector
    exps on ACT; the matrix branch2 is one 4x tensor_scalar per tile.
    Branch1 stays on ACT (exp with per-partition f2 bias). max+mask on DVE.
  - f1 is obtained for free as a 257th column of the x_user matmul
    (moving operand carries xw1 = xloc @ (W a1)).
  - attention accumulates 8 user-tile PSUM banks across loc tiles as PT
    tiles are produced (ACT/DVE/PE pipeline); Z is the 257th whext column.
  - one PSUM pool (8 bufs x 1 bank) rotated through all phases.
  - junk warm-up matmuls keep the PE HAM un-throttled through the input
    DMA window; day-0 loc block is written from SBUF (f16->f32 cast)
    instead of a DRAM->DRAM copy.

Sharding: unchanged -- 8 cores = 4 batch pairs, both cores of a pair run the
full per-batch recurrence; odd cores get user-rotated (by 512) index
tensors and each writes half of the user rows plus one of the two
(identical) loc blocks.
"""
import numpy as np

N_USER = 1024
N_LOC = 1024
DM = 256
HD = 256
B = 4
D = 5
ALPHA = 0.2
P = 128
NCORES = 8

# bundle free-dim offsets (f16 words)
OFF_XLT = 0              # xlocT16  [128, 2*1024]
OFF_XLE = 2048           # xloc16e  [128, 8*257]
OFF_W = OFF_XLE + 8 * 257       # W16   [128, 2*256]
OFF_WT = OFF_W + 512            # WT16  [128, 2*256]
OFF_AC = OFF_WT + 512           # acol16 [128, 4]
OFF_UW = OFF_AC + 4             # uw3   [128, 8*3]
NB = OFF_UW + 24

# smalls [3, *] free-dim offsets (day-indexed data lives on partition 0,
# because matmul operands need base partition 0/32/64)
SM_NOTHAS = 0            # [0:1, 0:1024]
SM_HASR = 1024           # [0:1, 1024 + dd*1024 : ...]  (3 days)
SM_NOTHASR = 4096        # [0:1, 4096 + dd*1024 : ...]
SM_NER = 7168            # [0:1, 7168 + dd*2 : +2]
SM_HCOL = 7174           # [0:3, 7174:7175]
SM_CPAIR = 7175          # [0:2, 7175:7176]
NS = 7176

_CACHE = {}


def _apply_tile_patch():
    import concourse.tile as tile
    from concourse.tile_sem_assignment import tick_to_sem

    if not getattr(tile.TileContext, "_drain_patched", False):
        def _patched(self, tick_clock, wait_clock):
            nc = self.nc
            gc = tick_clock.global_clock
            for proc, sem in self.sems.allocated().items():
                t = gc[proc]
                if t and t > 0:
                    nc.sync.nop().wait_op(sem, tick_to_sem(t, proc), "sem-ge")
            nc.sync.drain()
            nc.all_engine_barrier()
            popped = nc._tile_sem_poison_stack.pop()
            assert popped is self._sem_poison
            nc.clear_and_free_semaphores(list(self.sems.allocated().values()))
            nc.all_engine_barrier()

        tile.TileContext._drain_and_barrier = _patched
        tile.TileContext._drain_patched = True

    import json as _json
    import concourse.bass_utils as _bu
    import concourse.bass2jax as _b2j

    if not getattr(_bu, "_wait_split_patched", False):
        _orig_compile = _bu.compile_bir_kernel

        def _split_waits(bir_json):
            j = _json.loads(bir_json)
            nid = [0]
            for fn in j.get("functions", []):
                for bb in fn.get("blocks", []):
                    out = []
                    for inst in bb.get("instructions", []):
                        si = inst.get("sync_info") or {}
                        ow = si.get("on_wait") or []
                        if len(ow) > 1:
                            for w in ow[:-1]:
                                nid[0] += 1
                                out.append({
                                    "debug": inst.get("debug", 0),
                                    "engine": inst.get("engine", "SP"),
                                    "ins": [],
                                    "name": f"WSPL-{nid[0]}",
                                    "opcode": "NoOp",
                                    "outs": [],
                                    "sync_info": {"on_update": [],
                                                  "on_wait": [w]},
                                })
                            si["on_wait"] = [ow[-1]]
                        out.append(inst)
                    bb["instructions"] = out
            return _json.dumps(j).encode()

        def _patched_compile(bir_json, tmpdir, neff_name="file.neff"):
            return _orig_compile(_split_waits(bir_json), tmpdir,
                                 neff_name=neff_name)

        _bu.compile_bir_kernel = _patched_compile
        _b2j.compile_bir_kernel = _patched_compile
        _bu._wait_split_patched = True


def _build_nc():
    import contextlib
    import concourse.bass as bass
    import concourse.tile as tile
    from concourse import mybir

    _apply_tile_patch()
    f32 = mybir.dt.float32
    f16 = mybir.dt.float16
    AF = mybir.ActivationFunctionType
    OP = mybir.AluOpType

    nc = bass.Bass()

    d_small = nc.dram_tensor("smalls", [3, NS], f16, kind="ExternalInput")
    d_bund = nc.dram_tensor("bund", [P, NB], f16, kind="ExternalInput")
    d_Ah = nc.dram_tensor("AhT", [P, 8 * N_USER], f16, kind="ExternalInput")
    d_MT = nc.dram_tensor("MTt", [P, 8 * N_USER], f16, kind="ExternalInput")
    d_out = nc.dram_tensor("out", [D, 1536, HD], f32, kind="ExternalOutput")

    with tile.TileContext(nc) as tc:
        with contextlib.ExitStack() as ctx:
            persist = ctx.enter_context(tc.tile_pool(name="persist", bufs=1))
            work = ctx.enter_context(tc.tile_pool(name="work", bufs=1))
            ps = ctx.enter_context(
                tc.tile_pool(name="ps", bufs=8, space="PSUM"))

            def pst(nm, part=P, width=512):
                return ps.tile([part, width], f32, name=nm, tag="pb")

            # ---------------- consts + warmup ----------------
            onesrow16 = persist.tile([1, P], f16, name="onesrow16")
            nc.vector.memset(onesrow16[:], 1.0)
            onescol16 = persist.tile([P, 1], f16, name="onescol16")
            nc.vector.memset(onescol16[:], 1.0)
            one11_16 = persist.tile([1, 1], f16, name="one11_16")
            nc.vector.memset(one11_16[:], 1.0)
            l3 = persist.tile([P, 3], f16, name="l3")
            nc.vector.memset(l3[:, 0:2], 0.0)
            nc.vector.memset(l3[:, 2:3], 1.0)
            bm10 = persist.tile([P, 1], f32, name="bm10")
            nc.vector.memset(bm10[:], -10.0)
            bm5 = persist.tile([P, 1], f32, name="bm5")
            nc.vector.memset(bm5[:], -5.0)
            # ---------------- input DMAs (sync HWDGE, in order) ----------
            bund = persist.tile([P, NB], f16, name="bund")
            nc.sync.dma_start(out=bund[:], in_=d_bund[:])
            AhT = persist.tile([P, 8 * N_USER], f16, name="AhT")
            MTt = persist.tile([P, 8 * N_USER], f16, name="MTt")
            H = 4 * N_USER
            nc.sync.dma_start(out=AhT[:, 0:H], in_=d_Ah[:, 0:H])
            nc.sync.dma_start(out=AhT[:, H:2 * H], in_=d_Ah[:, H:2 * H])
            nc.sync.dma_start(out=MTt[:, 0:H], in_=d_MT[:, 0:H])
            nc.sync.dma_start(out=MTt[:, H:2 * H], in_=d_MT[:, H:2 * H])
            smalls = persist.tile([3, NS], f16, name="smalls")
            nc.sync.dma_start(out=smalls[:], in_=d_small[:])

            def xlt(kd, sl):
                return bund[:, OFF_XLT + kd * N_LOC:OFF_XLT + (kd + 1) * N_LOC][:, sl]

            def xle(lt, w=257):
                return bund[:, OFF_XLE + lt * 257:OFF_XLE + lt * 257 + w]

            def W16(kd):
                return bund[:, OFF_W + kd * HD:OFF_W + (kd + 1) * HD]

            def WT16(kd):
                return bund[:, OFF_WT + kd * DM:OFF_WT + (kd + 1) * DM]

            W16c16 = persist.tile([P, 4 * HD], f16, name="W16c16")

            def W16c(j):
                return W16c16[:, j * HD:(j + 1) * HD]

            acol16 = bund[:, OFF_AC:OFF_AC + 4]
            nothas = smalls[0:1, SM_NOTHAS:SM_NOTHAS + N_USER]

            # ---------------- phase 1: small matmuls off the bundle ------
            for kd in range(2):
                nc.vector.tensor_scalar(out=W16c(kd), in0=W16(kd),
                                        scalar1=2048.0 / 3072.0,
                                        scalar2=None, op0=OP.mult)
                nc.vector.tensor_scalar(out=W16c(2 + kd), in0=W16(kd),
                                        scalar1=1.0 / 3072.0,
                                        scalar2=None, op0=OP.mult)
            wacols_ps = pst("wacols")
            for j in range(4):          # j = ai*2 + dm-chunk
                ai, mt = j // 2, j % 2
                for kh in range(2):
                    nc.tensor.matmul(
                        wacols_ps[:, j:j + 1],
                        WT16(kh)[:, mt * P:(mt + 1) * P],
                        acol16[:, ai * 2 + kh:ai * 2 + kh + 1],
                        start=(kh == 0), stop=(kh == 1))
            wa16 = persist.tile([P, 4], f16, name="wa16")
            nc.scalar.copy(wa16[:], wacols_ps[:, 0:4])

            xwps = pst("xwps")
            for lt in range(8):
                sl = slice(lt * P, (lt + 1) * P)
                for kd in range(2):
                    nc.tensor.matmul(xwps[:, lt:lt + 1], xlt(kd, sl),
                                     wa16[:, kd:kd + 1],
                                     start=(kd == 0), stop=(kd == 1))
                for kd in range(2):
                    nc.tensor.matmul(xwps[:, 8 + lt:9 + lt], xlt(kd, sl),
                                     wa16[:, 2 + kd:3 + kd],
                                     start=(kd == 0), stop=(kd == 1))
            # insert xw1 columns into the 257-wide xloc tiles
            nc.scalar.copy(
                bund[:, OFF_XLE:OFF_XLE + 8 * 257]
                .rearrange("p (l w) -> p l w", l=8)[:, :, 256:257]
                .rearrange("p l w -> p (l w)"),
                xwps[:, 0:8])
            fb1c = persist.tile([P, 8], f32, name="fb1c")
            nc.scalar.activation(fb1c[:], xwps[:, 8:16], AF.Identity,
                                 bias=bm10[:])
            dcols = persist.tile([P, 8], f32, name="dcols")
            nc.scalar.activation(dcols[:], xwps[:, 8:16], AF.Exp,
                                 bias=bm5[:], scale=ALPHA)
            fb2c = persist.tile([P, 8], f32, name="fb2c")
            nc.scalar.activation(fb2c[:], xwps[:, 8:16], AF.Identity,
                                 bias=bm10[:], scale=ALPHA)

            whext = persist.tile([P, 8 * 257], f16, name="whext")
            nc.vector.memset(
                whext[:].rearrange("p (l w) -> p l w", l=8)[:, :, 256:257],
                1.0)
            for lt in range(8):
                sl = slice(lt * P, (lt + 1) * P)
                whp = pst(f"wh{lt}")
                for kd in range(2):
                    nc.tensor.matmul(whp[:, 0:HD], xlt(kd, sl), W16(kd),
                                     start=(kd == 0), stop=(kd == 1))
                nc.vector.tensor_copy(whext[:, lt * 257:lt * 257 + HD],
                                      whp[:, 0:HD])

            # day-0 loc block staging (f16 -> f32); DMA is issued after the
            # MT load so the 1MB write does not delay the mask transfer.
            xlocf32 = persist.tile([P, 8 * DM], f32, name="xlocf32")
            nc.vector.tensor_copy(
                xlocf32[:].rearrange("p (l w) -> p l w", l=8),
                bund[:, OFF_XLE:OFF_XLE + 8 * 257]
                .rearrange("p (l w) -> p l w", l=8)[:, :, 0:256])
            nc.sync.dma_start(
                out=d_out[0, 512:1536, :].rearrange("(t p) h -> p t h", p=P),
                in_=xlocf32[:].rearrange("p (t h) -> p t h", t=8))

            # ---------------- phase 2: x_user (+f1 column) ---------------
            # F1B built directly: stationary xw1 column broadcast across all
            # 128 array columns (stride-0 free dim) -> every out partition
            # gets f1_u = sum_l xw1_l * AhT[l, u]. No transposes, no f1row.
            xw1c16 = persist.tile([P, 8], f16, name="xw1c16")
            nc.scalar.copy(xw1c16[:], xwps[:, 0:8])
            F1B16 = persist.tile([P, N_USER], f16, name="F1B16")
            for c in range(2):
                fbp = pst(f"fbp{c}")
                for lt in range(8):
                    w = xw1c16[:, lt:lt + 1]
                    wb = bass.AP(tensor=w.tensor, offset=w.offset,
                                 ap=[list(w.ap[0]), [0, P]])
                    nc.tensor.matmul(
                        fbp[:],
                        wb,
                        AhT[:, lt * N_USER + c * 512:lt * N_USER + (c + 1) * 512],
                        start=(lt == 0), stop=(lt == 7))
                nc.vector.tensor_copy(F1B16[:, c * 512:(c + 1) * 512], fbp[:])
            E2B16 = persist.tile([P, N_USER], f16, name="E2B16")
            nc.scalar.activation(E2B16[:], F1B16[:], AF.Exp,
                                 bias=bm5[:], scale=ALPHA)

            xu_all = persist.tile([P, 4 * DM], f32, name="xu_all")
            xu16 = persist.tile([P, 8 * DM], f16, name="xu16")
            for ut in range(8):
                xp = pst(f"xu{ut}")
                for lt in range(8):
                    nc.tensor.matmul(
                        xp[:, 0:DM],
                        AhT[:, lt * N_USER + ut * P:lt * N_USER + (ut + 1) * P],
                        xle(lt, 256), start=(lt == 0), stop=(lt == 7))
                nc.vector.tensor_copy(xu16[:, ut * DM:(ut + 1) * DM],
                                      xp[:, 0:DM])


            # ---------------- phase 3: means / day-0 ---------------------
            stp = pst("st3", part=3)
            for ut in range(8):
                nc.tensor.matmul(stp[0:3, 0:DM],
                                 bund[:, OFF_UW + ut * 3:OFF_UW + (ut + 1) * 3],
                                 xu16[:, ut * DM:(ut + 1) * DM],
                                 start=(ut == 0), stop=False)
            for lt in range(8):
                nc.tensor.matmul(stp[0:3, 0:DM], l3[:], xle(lt, 256),
                                 start=False, stop=(lt == 7))
            stack16 = persist.tile([3, DM], f16, name="stack16")
            nc.scalar.copy(stack16[:], stp[0:3, 0:DM])
            mwe16 = stack16[0:1, :]

            # ---------------- phase 4: mw0 -------------------------------
            hcol16 = smalls[0:3, SM_HCOL:SM_HCOL + 1]
            mcp = pst("mcp")
            for mt in range(2):
                nc.tensor.matmul(mcp[:, mt:mt + 1],
                                 stack16[:, mt * P:(mt + 1) * P], hcol16,
                                 start=True, stop=True)
            mc16 = persist.tile([P, 2], f16, name="mc16")
            nc.scalar.copy(mc16[:], mcp[:, 0:2])
            mwp = pst("mwp", part=1)
            for kd in range(2):
                nc.tensor.matmul(mwp[0:1, 0:HD], mc16[:, kd:kd + 1], W16(kd),
                                 start=(kd == 0), stop=(kd == 1))
            mw0e16 = persist.tile([1, 257], f16, name="mw0e16")
            nc.vector.memset(mw0e16[:, 256:257], 1.0)
            nc.scalar.copy(mw0e16[:, 0:HD], mwp[0:1, 0:HD])
            # v1 = relu(mw0); early day-1 loc broadcast
            v1row16 = persist.tile([1, HD], f16, name="v1row16")
            nc.scalar.activation(v1row16[:], mwp[0:1, 0:HD], AF.Relu)

            def bcast_loc(day, vrow16, nm):
                bcp = pst(f"bc{nm}")
                nc.tensor.matmul(bcp[:, 0:HD], onesrow16[:], vrow16[:],
                                 start=True, stop=True)
                vst = work.tile([P, HD], f32, name=f"vst{nm}", tag="vst",
                                bufs=4)
                nc.vector.tensor_copy(vst[:], bcp[:, 0:HD])
                ap = vst[:]
                src = bass.AP(tensor=ap.tensor, offset=ap.offset,
                              ap=[list(ap.ap[0]), [0, 8], list(ap.ap[1])])
                dst = d_out[day, 512:1536, :].rearrange("(t p) h -> p t h", p=P)
                nc.sync.dma_start(out=dst, in_=src)

            bcast_loc(1, v1row16, "d1")

            # ---------------- phase 5+6: e-stage + attention pipeline ----
            PT = persist.tile([P, 8 * N_USER], f16, name="PT")
            att = [pst(f"att{ut}") for ut in range(8)]
            for pair in range(4):
                x1 = work.tile([P, 2 * N_USER], f16, name="x1", tag="x1",
                               bufs=3)
                t2 = work.tile([P, 2 * N_USER], f16, name="t2", tag="t2",
                               bufs=2)
                for h in range(2):
                    lt = 2 * pair + h
                    nc.scalar.activation(
                        x1[:, h * N_USER:(h + 1) * N_USER], F1B16[:], AF.Exp,
                        bias=fb1c[:, lt:lt + 1])
                    if pair < 2:
                        # ACT is idle early in the e-stage window; let it
                        # compute branch2 directly for the first tiles.
                        nc.scalar.activation(
                            t2[:, h * N_USER:(h + 1) * N_USER], F1B16[:],
                            AF.Exp, bias=fb2c[:, lt:lt + 1], scale=ALPHA)
                    else:
                        nc.vector.tensor_scalar(
                            out=t2[:, h * N_USER:(h + 1) * N_USER],
                            in0=E2B16[:], scalar1=dcols[:, lt:lt + 1],
                            scalar2=None, op0=OP.mult)
                mx = work.tile([P, 2 * N_USER], f16, name="mx", tag="mx",
                               bufs=2)
                nc.vector.tensor_tensor(out=mx[:], in0=x1[:], in1=t2[:],
                                        op=OP.max)
                nc.vector.tensor_tensor(
                    out=PT[:, 2 * pair * N_USER:(2 * pair + 2) * N_USER],
                    in0=mx[:],
                    in1=MTt[:, 2 * pair * N_USER:(2 * pair + 2) * N_USER],
                    op=OP.mult)
                for h in range(2):
                    lt = 2 * pair + h
                    for ut in range(8):
                        nc.tensor.matmul(
                            att[ut][:, 0:257],
                            PT[:, lt * N_USER + ut * P:lt * N_USER + (ut + 1) * P],
                            whext[:, lt * 257:(lt + 1) * 257],
                            start=(lt == 0), stop=False)

            for ut in range(8):
                nc.tensor.matmul(att[ut][:, 0:257],
                                 nothas[:, ut * P:(ut + 1) * P], mw0e16[:],
                                 start=False, stop=True)

            # finalize h1 = relu(numer/Z)
            h1u = persist.tile([P, 4 * DM], f32, name="h1u")
            h1u16 = persist.tile([P, 8 * DM], f16, name="h1u16")
            for ut in range(8):
                zr = work.tile([P, 1], f32, name="zr", tag="zr", bufs=3)
                nc.vector.reciprocal(zr[:], att[ut][:, 256:257])
                nc.scalar.activation(h1u16[:, ut * DM:(ut + 1) * DM],
                                     att[ut][:, 0:DM], AF.Relu, scale=zr[:])
                if ut < 4:
                    nc.vector.tensor_scalar(
                        out=h1u[:, ut * DM:(ut + 1) * DM],
                        in0=att[ut][:, 0:DM], scalar1=zr[:], scalar2=0.0,
                        op0=OP.mult, op1=OP.max)
            nc.sync.dma_start(
                out=d_out[1, 0:512, :].rearrange("(t p) h -> p t h", p=P),
                in_=h1u[:].rearrange("p (t h) -> p t h", t=4))

            # day-0 completion (off the critical path): stage f32 users,
            # add the nothas (x) mwe fallback, write day-0 user rows.
            nc.vector.tensor_copy(xu_all[:], xu16[:, 0:4 * DM])
            for c in range(2):
                mbp = pst(f"mb{c}")
                for h in range(2):
                    nc.tensor.matmul(
                        mbp[:, h * DM:(h + 1) * DM],
                        nothas[:, (2 * c + h) * P:(2 * c + h + 1) * P],
                        mwe16, start=True, stop=True)
                nc.vector.tensor_tensor(
                    out=xu_all[:, c * 512:(c + 1) * 512],
                    in0=xu_all[:, c * 512:(c + 1) * 512],
                    in1=mbp[:], op=OP.add)
            nc.sync.dma_start(
                out=d_out[0, 0:512, :].rearrange("(t p) h -> p t h", p=P),
                in_=xu_all[:].rearrange("p (t h) -> p t h", t=4))

            # ---------------- phase 7: days 2..4 row algebra -------------
            # vs16 cols: [v_kd0, s_kd0, v_kd1, s_kd1]
            vsp = pst("vs1")
            for mt in range(2):
                nc.tensor.matmul(vsp[:, 2 * mt:2 * mt + 1],
                                 v1row16[0:1, mt * P:(mt + 1) * P],
                                 one11_16[:], start=True, stop=True)
                for ut in range(8):
                    nc.tensor.matmul(
                        vsp[:, 2 * mt + 1:2 * mt + 2],
                        h1u16[:, ut * DM + mt * P:ut * DM + (mt + 1) * P],
                        onescol16[:], start=(ut == 0), stop=(ut == 7))
            vs16 = work.tile([P, 4], f16, name="vs16_1", tag="vs16", bufs=2)
            nc.scalar.copy(vs16[:], vsp[:, 0:4])

            for day in (2, 3, 4):
                dd = day - 2
                outu = work.tile([P, 4 * DM], f32, name=f"outu{day}",
                                 tag="outu", bufs=3)
                skp = pst(f"stk{day}", part=1)
                for kd in range(2):
                    nc.tensor.matmul(skp[0:1, 0:HD],
                                     vs16[:, 2 * kd:2 * kd + 1], W16(kd),
                                     start=(kd == 0), stop=(kd == 1))
                mrp = pst(f"mr{day}", part=1)
                for kd in range(2):
                    nc.tensor.matmul(mrp[0:1, 0:HD],
                                     vs16[:, 2 * kd:2 * kd + 1], W16c(kd),
                                     start=(kd == 0), stop=False)
                    nc.tensor.matmul(mrp[0:1, 0:HD],
                                     vs16[:, 2 * kd + 1:2 * kd + 2],
                                     W16c(2 + kd),
                                     start=False, stop=(kd == 1))
                r1_16 = work.tile([1, HD], f16, name=f"r1_{day}", tag="r1",
                                  bufs=2)
                nc.scalar.activation(r1_16[:], skp[0:1, 0:HD], AF.Relu)
                vn16 = work.tile([1, HD], f16, name=f"vn{day}", tag="vn",
                                 bufs=2)
                nc.scalar.activation(vn16[:], mrp[0:1, 0:HD], AF.Relu)
                bcast_loc(day, vn16, f"d{day}")
                for ut in range(4):
                    oup = pst(f"ou{day}_{ut}")
                    usl = slice(SM_HASR + dd * N_USER + ut * P,
                                SM_HASR + dd * N_USER + (ut + 1) * P)
                    nsl = slice(SM_NOTHASR + dd * N_USER + ut * P,
                                SM_NOTHASR + dd * N_USER + (ut + 1) * P)
                    nc.tensor.matmul(oup[:, 0:HD],
                                     smalls[0:1, usl], r1_16[:],
                                     start=True, stop=False)
                    nc.tensor.matmul(oup[:, 0:HD],
                                     smalls[0:1, nsl], vn16[:],
                                     start=False, stop=True)
                    nc.vector.tensor_copy(outu[:, ut * DM:(ut + 1) * DM],
                                          oup[:, 0:HD])
                nc.sync.dma_start(
                    out=d_out[day, 0:512, :].rearrange("(t p) h -> p t h", p=P),
                    in_=outu[:].rearrange("p (t h) -> p t h", t=4))
                if day < 4:
                    vnp = pst(f"vn{day}")
                    for mt in range(2):
                        nc.tensor.matmul(vnp[:, 2 * mt:2 * mt + 1],
                                         vn16[0:1, mt * P:(mt + 1) * P],
                                         one11_16[:], start=True, stop=True)
                        nc.tensor.matmul(
                            vnp[:, 2 * mt + 1:2 * mt + 2],
                            r1_16[0:1, mt * P:(mt + 1) * P],
                            smalls[0:1, SM_NER + 2 * dd:SM_NER + 2 * dd + 1],
                            start=True, stop=False)
                        nc.tensor.matmul(
                            vnp[:, 2 * mt + 1:2 * mt + 2],
                            vn16[0:1, mt * P:(mt + 1) * P],
                            smalls[0:1,
                                   SM_NER + 2 * dd + 1:SM_NER + 2 * dd + 2],
                            start=False, stop=True)
                    vs16 = work.tile([P, 4], f16, name=f"vs16_{day}",
                                     tag="vs16", bufs=2)
                    nc.scalar.copy(vs16[:], vnp[:, 0:4])
    return nc


def _tile128(x):
    r, c = x.shape
    return np.ascontiguousarray(
        x.reshape(r // P, P, c).transpose(1, 0, 2).reshape(P, -1))


def _host_prep(x_loc, mob_links, text_links, W, a):
    x_loc = np.asarray(x_loc, np.float32)
    W = np.asarray(W, np.float32)
    a = np.asarray(a, np.float32).reshape(-1)
    mob = np.asarray(mob_links)
    text = np.asarray(text_links)

    xe = np.zeros((N_LOC, 257), np.float32)
    xe[:, :256] = x_loc
    bund_shared = np.concatenate([
        _tile128(np.ascontiguousarray(x_loc.T)),
        _tile128(xe),
        _tile128(W),
        _tile128(np.ascontiguousarray(W.T)),
        a.reshape(2, 2, P).transpose(2, 0, 1).reshape(P, 4),
        np.zeros((P, 24), np.float32),   # per-core uw3 patched below
    ], axis=1).astype(np.float16)

    in_maps = []
    for c in range(NCORES):
        b, r = c // 2, c % 2
        rot = r * 512
        u0 = np.concatenate([mob[b, 0, :, 0], text[b, 0, :, 0]]).astype(np.int64)
        l0 = np.concatenate([mob[b, 0, :, 1], text[b, 0, :, 1]]).astype(np.int64)
        cnt = np.bincount(u0, minlength=N_USER).astype(np.float32)
        A = np.zeros((N_USER, N_LOC), np.float32)
        np.add.at(A, (u0, l0), 1.0)
        Ahat = A / np.maximum(cnt, 1.0)[:, None]
        Mb = np.zeros((N_USER, N_LOC), np.float32)
        Tb = np.zeros((N_USER, N_LOC), np.float32)
        Mb[mob[b, 0, :, 0], mob[b, 0, :, 1]] = 1.0
        Tb[text[b, 0, :, 0], text[b, 0, :, 1]] = 1.0
        M = Mb + Tb
        has0 = (cnt > 0).astype(np.float32)
        n_with = max(float(has0.sum()), 1.0)
        nh_cnt = float(N_USER) - float(has0.sum())

        def rollu(x, axis=0):
            return np.roll(x, -rot, axis=axis)

        hasE = np.zeros((3, N_USER), np.float32)
        for dd in range(3):
            us = np.concatenate([mob[b, dd + 1, :, 0], text[b, dd + 1, :, 0]])
            hasE[dd, us] = 1.0
        hasr = np.stack([rollu(hasE[dd]) for dd in range(3)])
        ner = np.stack([np.array([hasE[dd].sum(), N_USER - hasE[dd].sum()],
                                 np.float32) for dd in range(3)])
        nothas_r = rollu(1.0 - has0)
        hw = (rollu(has0) / n_with).astype(np.float32)
        uw3 = np.stack([hw, np.ones(N_USER, np.float32),
                        np.zeros(N_USER, np.float32)], axis=1)

        smalls = np.zeros((3, NS), np.float32)
        smalls[0, SM_NOTHAS:SM_NOTHAS + N_USER] = nothas_r
        smalls[0, SM_HASR:SM_HASR + 3 * N_USER] = hasr.reshape(-1)
        smalls[0, SM_NOTHASR:SM_NOTHASR + 3 * N_USER] = \
            (1.0 - hasr).reshape(-1)
        smalls[0, SM_NER:SM_NER + 6] = ner.reshape(-1)
        smalls[0:3, SM_HCOL:SM_HCOL + 1] = np.array(
            [[nh_cnt / 3072.0], [1.0 / 3072.0], [2.0 / 3072.0]], np.float32)
        smalls[0:2, SM_CPAIR:SM_CPAIR + 1] = np.array(
            [[2048.0 / 3072.0], [1.0 / 3072.0]], np.float32)

        bund = bund_shared.copy()
        bund[:, OFF_UW:OFF_UW + 24] = _tile128(rollu(uw3)).astype(np.float16)
        m = {
            "smalls": smalls.astype(np.float16),
            "bund": bund,
            "AhT": _tile128(
                np.ascontiguousarray(rollu(Ahat, 0).T)).astype(np.float16),
            "MTt": _tile128(
                np.ascontiguousarray(rollu(M, 0).T)).astype(np.float16),
        }
        in_maps.append(m)
    return in_maps


def kernel(**inputs):
    from concourse.bass_utils import run_bass_kernel_spmd

    if "nc" not in _CACHE:
        _CACHE["nc"] = _build_nc()
    nc = _CACHE["nc"]

    in_maps = _host_prep(inputs["x_loc"], inputs["mob_links"],
                         inputs["text_links"], inputs["W"], inputs["a"])
    res = run_bass_kernel_spmd(nc, in_maps, core_ids=list(range(NCORES)))

    out = np.zeros((B, D, N_USER + 2 * N_LOC, HD), np.float32)
    for c in range(NCORES):
        b, r = c // 2, c % 2
        o = res.results[c]["out"]
        out[b, :, r * 512:(r + 1) * 512, :] = o[:, 0:512, :]
        out[b, :, N_USER + r * N_LOC:N_USER + (r + 1) * N_LOC, :] = \
            o[:, 512:1536, :]
    return out
